# revision 15
# baseline (speedup 1.0000x reference)
"""Corr1d (stereo cost volume) Trainium2 kernel, v5.

corrmap[b, i, h, w] = sum_c fL[b, c, h, w] * fR[b, c, h, w - i],  i in [0, 64)
Shapes: fL, fR [8, 128, 160, 320] f32 -> corrmap [8, 64, 160, 320] f32.
Sharding: data-parallel over batch; core k handles batch element k.
Host: f32->bf16 (RTNE) before upload; bf16 on device; upcast f32 on host.

v5 vs v2 (256us baseline).  Trace analysis showed every DMA queue is
latency-bound: throughput ~= 16 engines x run_bytes / ~600ns, so run length
is everything (1KB readback runs -> 58GB/s, 3KB dump runs -> 80GB/s, 10KB
load runs -> 282GB/s).  Changes:
  * h16 interleave: band rows store (g, n, h16) with all 16 h rows of the
    batch interleaved per band column -> diagonal readback runs are
    (i,h16) = 2KB (vs 1KB), dump runs 9KB (vs 3KB).
  * One dump per batch ([[32*GP5-512,4],[GP5,32],[1,4608]]): per-w-tile
    -512 element shift makes the readback address affine in partition p:
    3 readbacks, one per group, 2-dim APs [[GP5-16,P],[1,1024]].
    g2's matmuls sit at PSUM partitions 64-127 (tile_position=(0,64/96)).
  * Rings: loads on gpsimd, dump+readbacks on sync (FIFO ordering for the
    DRAM round trip), stores split scalar/gpsimd.
  * Transposes: 8 chunks of (i8, h16) per group; stores per i8-chunk
    [[HW,8],[W,16],[1,W]].

Self-contained: shapes hardcoded; requires only numpy + ml_dtypes + concourse.
"""

import ml_dtypes
import numpy as np

import concourse.bacc as bacc
import concourse.bass as bass
import concourse.mybir as mybir
from concourse.bass_utils import run_bass_kernel_spmd
from concourse.tile import TileContext
from concourse.masks import make_identity

F32 = mybir.dt.float32
BF16 = mybir.dt.bfloat16

N_CORES = 8
C = 128           # channels (matmul contraction dim)
H = 160
W = 320
D = 64            # disparities
NH = 16           # h rows per batch
NB = H // NH      # batches (10)
NS = 95           # band columns per 32-wide w-tile (32 + 63)
SC = 96           # stored band columns per tile row
FRPAD = 64        # zero pad columns at the start of the fR buffer
HW = H * W
GR = SC * NH      # 1536: (n, h16) elements per (p, group)
BROW = 3 * GR     # 4608: (g, n, h16) per (p, batch)
GP5 = NB * BROW + 512 * 3   # 47616: scratch row pitch (+1536 shift margin)

_cache = {}


def _build():
    nc = bacc.Bacc("TRN2", target_bir_lowering=False, debug=False,
                   num_devices=N_CORES)
    fL = nc.dram_tensor("fL", [C, H, W], BF16, kind="ExternalInput")
    fR = nc.dram_tensor("fR", [C, H, W], BF16, kind="ExternalInput")
    out = nc.dram_tensor("out", [D, H, W], BF16, kind="ExternalOutput")
    scratch = nc.dram_tensor("scratch", [128, GP5], BF16)

    with TileContext(nc) as tc:
        NLB = 3
        fLb = [nc.alloc_sbuf_tensor(f"fLb{i}", [C, NH * W], BF16)
               for i in range(NLB)]
        fRb = [nc.alloc_sbuf_tensor(f"fRb{i}", [C, FRPAD + NH * W], BF16)
               for i in range(NLB)]
        ident = nc.alloc_sbuf_tensor("ident", [128, 128], BF16)
        make_identity(nc, ident.ap())
        for i in range(NLB):
            nc.vector.memset(fRb[i].ap()[:, 0:FRPAD], 0.0)

        with (
            tc.tile_pool(name="sb", bufs=3) as pool,
            tc.tile_pool(name="ps", bufs=2, space="PSUM") as pp,
        ):
            def emit_loads(b):
                li, ri = fLb[b % NLB], fRb[b % NLB]
                h0 = b * NH
                nc.gpsimd.dma_start(
                    out=li.ap(),
                    in_=bass.AP(fL, h0 * W, [[HW, C], [1, NH * W]]),
                )
                nc.gpsimd.dma_start(
                    out=ri.ap()[:, FRPAD:],
                    in_=bass.AP(fR, h0 * W, [[HW, C], [1, NH * W]]),
                )

            def emit_front(b):
                # matmuls + psum->band copies + garbage memsets + dump +
                # readbacks for batch b; returns the T tiles.
                li, ri = fLb[b % NLB], fRb[b % NLB]
                # band: [128, (g, n, h16)]
                bnd = pool.tile([128, BROW], BF16, tag="bnd", name=f"bnd_{b}")
                pitch = BROW
                for hq in range(NH // 4):
                    pss = []
                    for g in range(3):
                        # g2 tiles live at PSUM/band partitions 64-127
                        P0 = 64 if g == 2 else 0
                        nt = 2 if g == 2 else 4
                        ps = pp.tile([128, 4 * NS], F32, tag=f"ps{g}",
                                     name=f"ps{g}_{b}_{hq}",
                                     padded_shape=[128, 512])
                        pss.append(ps)
                        for j4 in range(4):
                            hh = 4 * hq + j4
                            for q in range(nt):
                                wt = g * 128 + 32 * q
                                cp = P0 + 32 * q
                                lhsT = bass.AP(li, hh * W + wt,
                                               [[NH * W, C], [1, 32]])
                                rhs = bass.AP(ri, FRPAD + hh * W + wt + 31,
                                              [[FRPAD + NH * W, C], [-1, NS]])
                                nc.tensor.matmul(
                                    ps[cp:cp + 32,
                                       j4 * NS:(j4 + 1) * NS],
                                    lhsT, rhs, start=True, stop=True,
                                    tile_position=(0, cp),
                                )
                    for g in range(3):
                        P0 = 64 if g == 2 else 0
                        P = 64 if g == 2 else 128
                        base = g * GR + 4 * hq
                        o = bass.AP(bnd.tensor, P0 * pitch + base,
                                    [[pitch, P], [NH, NS], [1, 4]])
                        pp_pitch = pss[g].tensor.shape[-1]
                        i_ = bass.AP(pss[g].tensor, P0 * pp_pitch,
                                     [[pp_pitch, P], [1, NS], [NS, 4]])
                        if (hq + g) % 2 == 0:
                            nc.vector.tensor_copy(out=o, in_=i_)
                        else:
                            nc.scalar.copy(o, i_)
                # zero the never-written pad column n=95 of each group
                nc.vector.memset(
                    bass.AP(bnd.tensor, NS * NH,
                            [[pitch, 128], [GR, 3], [1, NH]]),
                    0.0)
                # zero i > w zones (g0 tiles 0 and 1): band cols n>=32 / n>=64
                nc.vector.memset(
                    bass.AP(bnd.tensor, 32 * NH,
                            [[pitch, 32], [1, (SC - 32) * NH]]),
                    0.0)
                nc.vector.memset(
                    bass.AP(bnd.tensor, 32 * pitch + 64 * NH,
                            [[pitch, 32], [1, (SC - 64) * NH]]),
                    0.0)

                # dumps: rows shifted by -512 elements per w-tile index.
                # One DMA per w-tile q keeps the DRAM-side outer dim at 32 so
                # the HWDGE sprays descriptors across all 16 SDMA engines
                # (the spray follows the DRAM AP's outermost dim count), and
                # the SBUF side stays a plain contiguous partition range
                # (partition crossing is only legal in AP dim 0).
                for q in range(4):
                    # partitions 0-63 (tiles q<2) never write the g2 region;
                    # dump only (g0, g1) there.
                    ext = BROW if q >= 2 else 2 * GR
                    nc.sync.dma_start(
                        out=bass.AP(scratch,
                                    b * BROW + q * (32 * GP5 - 512),
                                    [[GP5, 32], [1, ext]]),
                        in_=bnd[32 * q:32 * q + 32, 0:ext],
                    )
                # diagonal readbacks: T[p, i*16 + h16]
                # addr = p*(GP5-16) + b*BROW + g*GR + 31*16 + i*16 + h16
                Ts = []
                for g in range(3):
                    P0 = 64 if g == 2 else 0
                    P = 64 if g == 2 else 128
                    T = pool.tile([P, D * NH], BF16, tag=f"T{g}",
                                  name=f"T{g}_{b}")
                    Ts.append(T)
                    nc.sync.dma_start(
                        out=T[:, :],
                        in_=bass.AP(scratch,
                                    P0 * (GP5 - 16) + b * BROW
                                    + g * GR + 31 * NH,
                                    [[GP5 - 16, P], [1, D * NH]]),
                    )
                return Ts

            def emit_back(b, Ts):
                # transposes + staging copies + output DMAs for batch b
                stg = pool.tile([128, 8 * W], BF16, tag="stg",
                                name=f"stg_{b}")
                for a in range(8):
                    u = pp.tile([128, W], BF16, tag="U", name=f"U_{b}_{a}",
                                padded_shape=[128, 1024])
                    cs = 128 * a
                    nc.tensor.transpose(
                        u[:, 0:128], Ts[0][:, cs:cs + 128], ident.ap())
                    nc.tensor.transpose(
                        u[:, 128:256], Ts[1][:, cs:cs + 128], ident.ap())
                    nc.tensor.transpose(
                        u[:, 256:320], Ts[2][:, cs:cs + 128],
                        ident.ap()[0:64, 0:64])
                    o = stg[:, a * W:(a + 1) * W]
                    if a % 2 == 0:
                        nc.vector.tensor_copy(out=o, in_=u[:, :])
                    else:
                        nc.scalar.copy(o, u[:, :])
                # SWDGE (gpsimd) concats the 640B w-run descriptors into
                # ~4.4KB packets; HWDGE leaves them at 640B.
                for a in range(8):
                    eng = nc.gpsimd if a % 2 == 0 else nc.scalar
                    eng.dma_start(
                        out=bass.AP(out, 8 * a * HW + NH * b * W,
                                    [[HW, 8], [W, NH], [1, W]]),
                        in_=bass.AP(stg.tensor, a * W,
                                    [[8 * W, 128], [1, W]]),
                    )

            # software pipeline: loads two batches ahead; front(b) is emitted
            # before back(b-1) so PE's in-order queue runs batch-b matmuls
            # (deps: loads only) before batch-(b-1) transposes (deps:
            # readback) -- PE never stalls on the DRAM round trip.
            emit_loads(0)
            emit_loads(1)
            prev = None
            for b in range(NB):
                if b + 2 < NB:
                    emit_loads(b + 2)
                cur = emit_front(b)
                if prev is not None:
                    emit_back(b - 1, prev)
                prev = cur
            emit_back(NB - 1, prev)

    nc.compile()
    return nc


def _make_in_maps(inputs: dict) -> list:
    fL = np.asarray(inputs["fL"], dtype=np.float32).astype(ml_dtypes.bfloat16)
    fR = np.asarray(inputs["fR"], dtype=np.float32).astype(ml_dtypes.bfloat16)
    fL = np.ascontiguousarray(fL)
    fR = np.ascontiguousarray(fR)
    return [{"fL": fL[k], "fR": fR[k]} for k in range(N_CORES)]


def kernel(fL: np.ndarray, fR: np.ndarray) -> np.ndarray:
    if "nc" not in _cache:
        _cache["nc"] = _build()
    nc = _cache["nc"]

    in_maps = _make_in_maps({"fL": fL, "fR": fR})
    res = run_bass_kernel_spmd(nc, in_maps, core_ids=list(range(N_CORES)))
    out = np.stack(
        [res.results[k]["out"].astype(np.float32) for k in range(N_CORES)],
        axis=0,
    )
    return out


if __name__ == "__main__":
    rng = np.random.default_rng(0)
    a = rng.standard_normal((N_CORES, C, H, W)).astype(np.float32)
    b = rng.standard_normal((N_CORES, C, H, W)).astype(np.float32)
    o = kernel(a, b)
    print("kernel ran, output shape", o.shape)


# revision 17
# speedup vs baseline: 1.0576x; 1.0576x over previous
"""Corr1d (stereo cost volume) Trainium2 kernel, v5.

corrmap[b, i, h, w] = sum_c fL[b, c, h, w] * fR[b, c, h, w - i],  i in [0, 64)
Shapes: fL, fR [8, 128, 160, 320] f32 -> corrmap [8, 64, 160, 320] f32.
Sharding: data-parallel over batch; core k handles batch element k.
Host: f32->bf16 (RTNE) before upload; bf16 on device; upcast f32 on host.

v5 vs v2 (256us baseline).  Trace analysis showed every DMA queue is
latency-bound: throughput ~= 16 engines x run_bytes / ~600ns, so run length
is everything (1KB readback runs -> 58GB/s, 3KB dump runs -> 80GB/s, 10KB
load runs -> 282GB/s).  Changes:
  * h16 interleave: band rows store (g, n, h16) with all 16 h rows of the
    batch interleaved per band column -> diagonal readback runs are
    (i,h16) = 2KB (vs 1KB), dump runs 9KB (vs 3KB).
  * One dump per batch ([[32*GP5-512,4],[GP5,32],[1,4608]]): per-w-tile
    -512 element shift makes the readback address affine in partition p:
    3 readbacks, one per group, 2-dim APs [[GP5-16,P],[1,1024]].
    g2's matmuls sit at PSUM partitions 64-127 (tile_position=(0,64/96)).
  * Rings: loads on gpsimd, dump+readbacks on sync (FIFO ordering for the
    DRAM round trip), stores split scalar/gpsimd.
  * Transposes: 8 chunks of (i8, h16) per group; stores per i8-chunk
    [[HW,8],[W,16],[1,W]].

Self-contained: shapes hardcoded; requires only numpy + ml_dtypes + concourse.
"""

import ml_dtypes
import numpy as np

import concourse.bacc as bacc
import concourse.bass as bass
import concourse.mybir as mybir
from concourse.bass_utils import run_bass_kernel_spmd
from concourse.tile import TileContext
from concourse.masks import make_identity

F32 = mybir.dt.float32
BF16 = mybir.dt.bfloat16

N_CORES = 8
C = 128           # channels (matmul contraction dim)
H = 160
W = 320
D = 64            # disparities
NH = 16           # h rows per batch
NB = H // NH      # batches (10)
NS = 95           # band columns per 32-wide w-tile (32 + 63)
SC = 96           # stored band columns per tile row
FRPAD = 64        # zero pad columns at the start of the fR buffer
HW = H * W
GR = SC * NH      # 1536: (n, h16) elements per (p, group)
BROW = 3 * GR     # 4608: (g, n, h16) per (p, batch)
GP5 = NB * BROW + 512 * 3   # 47616: scratch row pitch (+1536 shift margin)

_cache = {}


def _build():
    nc = bacc.Bacc("TRN2", target_bir_lowering=False, debug=False,
                   num_devices=N_CORES)
    fL = nc.dram_tensor("fL", [C, H, W], BF16, kind="ExternalInput")
    fR = nc.dram_tensor("fR", [C, H, W], BF16, kind="ExternalInput")
    out = nc.dram_tensor("out", [D, H, W], BF16, kind="ExternalOutput")
    scratch = nc.dram_tensor("scratch", [128, GP5], BF16)

    with TileContext(nc) as tc:
        NLB = 3
        fLb = [nc.alloc_sbuf_tensor(f"fLb{i}", [C, NH * W], BF16)
               for i in range(NLB)]
        fRb = [nc.alloc_sbuf_tensor(f"fRb{i}", [C, FRPAD + NH * W], BF16)
               for i in range(NLB)]
        ident = nc.alloc_sbuf_tensor("ident", [128, 128], BF16)
        make_identity(nc, ident.ap())
        for i in range(NLB):
            nc.vector.memset(fRb[i].ap()[:, 0:FRPAD], 0.0)

        with (
            tc.tile_pool(name="sb", bufs=3) as pool,
            tc.tile_pool(name="ps", bufs=2, space="PSUM") as pp,
        ):
            def emit_loads(b):
                li, ri = fLb[b % NLB], fRb[b % NLB]
                h0 = b * NH
                nc.gpsimd.dma_start(
                    out=li.ap(),
                    in_=bass.AP(fL, h0 * W, [[HW, C], [1, NH * W]]),
                )
                nc.gpsimd.dma_start(
                    out=ri.ap()[:, FRPAD:],
                    in_=bass.AP(fR, h0 * W, [[HW, C], [1, NH * W]]),
                )

            def emit_front(b):
                # matmuls + psum->band copies + garbage memsets + dump +
                # readbacks for batch b; returns the T tiles.
                li, ri = fLb[b % NLB], fRb[b % NLB]
                # band: [128, (g, n, h16)]
                bnd = pool.tile([128, BROW], BF16, tag="bnd", name=f"bnd_{b}")
                pitch = BROW
                for hq in range(NH // 4):
                    pss = []
                    for g in range(3):
                        # g2 tiles live at PSUM/band partitions 64-127
                        P0 = 64 if g == 2 else 0
                        nt = 2 if g == 2 else 4
                        ps = pp.tile([128, 4 * NS], F32, tag=f"ps{g}",
                                     name=f"ps{g}_{b}_{hq}",
                                     padded_shape=[128, 512])
                        pss.append(ps)
                        for j4 in range(4):
                            hh = 4 * hq + j4
                            for q in range(nt):
                                wt = g * 128 + 32 * q
                                cp = P0 + 32 * q
                                lhsT = bass.AP(li, hh * W + wt,
                                               [[NH * W, C], [1, 32]])
                                rhs = bass.AP(ri, FRPAD + hh * W + wt + 31,
                                              [[FRPAD + NH * W, C], [-1, NS]])
                                nc.tensor.matmul(
                                    ps[cp:cp + 32,
                                       j4 * NS:(j4 + 1) * NS],
                                    lhsT, rhs, start=True, stop=True,
                                    tile_position=(0, cp),
                                )
                    for g in range(3):
                        P0 = 64 if g == 2 else 0
                        P = 64 if g == 2 else 128
                        base = g * GR + 4 * hq
                        o = bass.AP(bnd.tensor, P0 * pitch + base,
                                    [[pitch, P], [NH, NS], [1, 4]])
                        pp_pitch = pss[g].tensor.shape[-1]
                        i_ = bass.AP(pss[g].tensor, P0 * pp_pitch,
                                     [[pp_pitch, P], [1, NS], [NS, 4]])
                        nc.vector.tensor_copy(out=o, in_=i_)
                # zero the never-written pad column n=95 of each group
                nc.vector.memset(
                    bass.AP(bnd.tensor, NS * NH,
                            [[pitch, 128], [GR, 3], [1, NH]]),
                    0.0)
                # zero i > w zones (g0 tiles 0 and 1): band cols n>=32 / n>=64
                nc.vector.memset(
                    bass.AP(bnd.tensor, 32 * NH,
                            [[pitch, 32], [1, (SC - 32) * NH]]),
                    0.0)
                nc.vector.memset(
                    bass.AP(bnd.tensor, 32 * pitch + 64 * NH,
                            [[pitch, 32], [1, (SC - 64) * NH]]),
                    0.0)

                # dumps: rows shifted by -512 elements per w-tile index.
                # One DMA per w-tile q keeps the DRAM-side outer dim at 32 so
                # the HWDGE sprays descriptors across all 16 SDMA engines
                # (the spray follows the DRAM AP's outermost dim count), and
                # the SBUF side stays a plain contiguous partition range
                # (partition crossing is only legal in AP dim 0).
                for q in range(4):
                    # partitions 0-63 (tiles q<2) never write the g2 region;
                    # dump only (g0, g1) there.
                    ext = BROW if q >= 2 else 2 * GR
                    nc.sync.dma_start(
                        out=bass.AP(scratch,
                                    b * BROW + q * (32 * GP5 - 512),
                                    [[GP5, 32], [1, ext]]),
                        in_=bnd[32 * q:32 * q + 32, 0:ext],
                    )
                return None

            def emit_readbacks(b):
                # diagonal readbacks: T[p, i*16 + h16]
                # addr = p*(GP5-16) + b*BROW + g*GR + 31*16 + i*16 + h16
                # Issued one batch late so the sync ring never sits on the
                # dump's completion-receipt bubble.
                Ts = []
                for g in range(3):
                    P0 = 64 if g == 2 else 0
                    P = 64 if g == 2 else 128
                    T = pool.tile([P, D * NH], BF16, tag=f"T{g}",
                                  name=f"T{g}_{b}")
                    Ts.append(T)
                    nc.sync.dma_start(
                        out=T[:, :],
                        in_=bass.AP(scratch,
                                    P0 * (GP5 - 16) + b * BROW
                                    + g * GR + 31 * NH,
                                    [[GP5 - 16, P], [1, D * NH]]),
                    )
                return Ts

            def emit_back(b, Ts):
                # transposes + staging copies + output DMAs for batch b
                stg = pool.tile([128, 8 * W], BF16, tag="stg",
                                name=f"stg_{b}")
                for a in range(8):
                    u = pp.tile([128, W], BF16, tag="U", name=f"U_{b}_{a}",
                                padded_shape=[128, 1024])
                    cs = 128 * a
                    nc.tensor.transpose(
                        u[:, 0:128], Ts[0][:, cs:cs + 128], ident.ap())
                    nc.tensor.transpose(
                        u[:, 128:256], Ts[1][:, cs:cs + 128], ident.ap())
                    nc.tensor.transpose(
                        u[:, 256:320], Ts[2][:, cs:cs + 128],
                        ident.ap()[0:64, 0:64])
                    o = stg[:, a * W:(a + 1) * W]
                    nc.scalar.copy(o, u[:, :])
                for a in range(8):
                    eng = nc.scalar
                    eng.dma_start(
                        out=bass.AP(out, 8 * a * HW + NH * b * W,
                                    [[HW, 8], [W, NH], [1, W]]),
                        in_=bass.AP(stg.tensor, a * W,
                                    [[8 * W, 128], [1, W]]),
                    )

            # software pipeline: loads two ahead; readback(b) issued during
            # iteration b+1 (ring always has ready work queued -- no
            # completion-receipt bubble); transposes+stores two behind, so
            # the PE stream is [matmuls b][transposes b-2] with all deps
            # long settled.
            emit_loads(0)
            emit_loads(1)
            Tmap = {}
            for b in range(NB):
                if b + 2 < NB:
                    emit_loads(b + 2)
                emit_front(b)
                if b >= 1:
                    Tmap[b - 1] = emit_readbacks(b - 1)
                if b >= 2:
                    emit_back(b - 2, Tmap.pop(b - 2))
            Tmap[NB - 1] = emit_readbacks(NB - 1)
            emit_back(NB - 2, Tmap.pop(NB - 2))
            emit_back(NB - 1, Tmap.pop(NB - 1))

    nc.compile()
    return nc


def _make_in_maps(inputs: dict) -> list:
    fL = np.asarray(inputs["fL"], dtype=np.float32).astype(ml_dtypes.bfloat16)
    fR = np.asarray(inputs["fR"], dtype=np.float32).astype(ml_dtypes.bfloat16)
    fL = np.ascontiguousarray(fL)
    fR = np.ascontiguousarray(fR)
    return [{"fL": fL[k], "fR": fR[k]} for k in range(N_CORES)]


def kernel(fL: np.ndarray, fR: np.ndarray) -> np.ndarray:
    if "nc" not in _cache:
        _cache["nc"] = _build()
    nc = _cache["nc"]

    in_maps = _make_in_maps({"fL": fL, "fR": fR})
    res = run_bass_kernel_spmd(nc, in_maps, core_ids=list(range(N_CORES)))
    out = np.stack(
        [res.results[k]["out"].astype(np.float32) for k in range(N_CORES)],
        axis=0,
    )
    return out


if __name__ == "__main__":
    rng = np.random.default_rng(0)
    a = rng.standard_normal((N_CORES, C, H, W)).astype(np.float32)
    b = rng.standard_normal((N_CORES, C, H, W)).astype(np.float32)
    o = kernel(a, b)
    print("kernel ran, output shape", o.shape)
